# revision 29
# baseline (speedup 1.0000x reference)
"""Trainium2 Bass kernel for nn_AttentionToVec (B=8, N=4096, E=1024, H=16, D=64).

Strategy: pure data-parallel over batch (1 batch element per NeuronCore), NO
collectives.  Each core computes its own row's full MLP with the complete
W1/W2 (profiling showed the AllGather/ReduceScatter + cc-barrier of the
tensor-parallel MLP cost ~100us, far more than the extra weight traffic).

DMA discipline: all loads go through the single HWDGE sync queue, which
drains FIFO in issue order.  Every stream tensor is fully resident in SBUF
(no pool-buffer gating), so the issue order IS the arrival order:
  xT (fp8, 4.2MB) -> x (fp8, 4.2MB) -> Wv (fp16, 2.1MB) -> W1 (fp8, 4.2MB)
  -> W2 (fp16, 8.4MB, sliced so the W2 matmuls pipeline behind arrival).
The W2 buffer reuses the xT pool's SBUF (xT pool closes after phase A).

Dtypes (validated vs reference on host, rel-err ~1.25e-2 vs 2e-2 budget):
  xT/x fp8e4m3 (mixed with fp16 operands), W1 fp8e4m3 (mixed with fp16
  moving s), everything else fp16; all matmul accumulation fp32 in PSUM.

Algebra (host does weight-only folding):
  - att logits = x @ w_att,  w_att[e,h] = sum_d W_k[e, h*D+d] * query[h,d]
    (the k-projection bias cancels inside softmax over n).
  - y[h,:] = sum_n exp_att[n,h] * x[n,:]  (deferred 1/Z normalization)
  - sampled[e] = (y[h(e),:] @ W_v[:, e]) + b_v[e],  h(e)=e//D.  Phase C
    computes ONLY the needed diagonal blocks, directly transposed:
    sfT_j[m, i] = sf[2j+i, 128j+m] so s[128j+m] = sfT_j[m, m//64].
  - MLP per-core on its own row, hidden laid out as zT[p, q] = z[128q+p]
    so gelu runs across all 128 partitions.
"""

import numpy as np

B = 8
N = 4096
E = 1024
H = 16
D = 64
HID = 4096
NCORES = 8

_CACHE = {}


def _build():
    import concourse.bacc as bacc
    import concourse.mybir as mybir
    from concourse import tile
    from concourse.masks import make_identity

    f32 = mybir.dt.float32
    f16 = mybir.dt.float16
    f8 = mybir.dt.float8e4
    Act = mybir.ActivationFunctionType
    Alu = mybir.AluOpType

    nc = bacc.Bacc(None, target_bir_lowering=False, debug=True, num_devices=NCORES)

    xT8 = nc.dram_tensor("xT8", [E, N], f8, kind="ExternalInput")
    x8 = nc.dram_tensor("x8", [N, E], f8, kind="ExternalInput")
    watt = nc.dram_tensor("watt", [E, H], f16, kind="ExternalInput")
    # packed [128, 80] f32: cols 0:32 maskn, 32:40 bvT, 40:72 b1T, 72:80 b2T
    cpack = nc.dram_tensor("cpack", [128, 80], f32, kind="ExternalInput")
    wv = nc.dram_tensor("wv", [E, E], f16, kind="ExternalInput")
    w1 = nc.dram_tensor("w1", [E, HID], f8, kind="ExternalInput")
    w2 = nc.dram_tensor("w2", [HID, E], f16, kind="ExternalInput")
    # outT[p, j] = out_row[128*j + p]; host reassembles
    out = nc.dram_tensor("out", [128, 8], f32, kind="ExternalOutput")

    with tile.TileContext(nc) as tc:
        with (
            tc.tile_pool(name="consts", bufs=1) as consts,
            tc.tile_pool(name="xp", bufs=1) as xp,
            tc.tile_pool(name="xtp", bufs=1) as xtp,
            tc.tile_pool(name="wvp", bufs=1) as wvp,
            tc.tile_pool(name="w2p", bufs=1) as w2p,
            tc.tile_pool(name="w1p", bufs=1) as w1p,
            tc.tile_pool(name="work", bufs=1) as work,
        ):
            identity = consts.tile([H, H], f16)
            make_identity(nc, identity[:])
            ones_s = consts.tile([128, 1], f16)
            nc.vector.memset(ones_s[:], 1.0)

            # ---- all DMA triggers in FIFO priority order ----
            watt_s = consts.tile([128, 8, H], f16)
            nc.sync.dma_start(
                out=watt_s[:], in_=watt.ap().rearrange("(c p) h -> p c h", p=128)
            )
            cp_s = consts.tile([128, 80], f32)
            nc.sync.dma_start(out=cp_s[:], in_=cpack[:, :])

            xT_s = xtp.tile([128, 8, N], f8)
            xTr = xT8.ap().rearrange("(c p) n -> c p n", p=128)
            nc.sync.dma_start(out=xT_s[:, 0, 0:2048], in_=xTr[0][:, 0:2048])
            nc.sync.dma_start(out=xT_s[:, 0, 2048:4096], in_=xTr[0][:, 2048:4096])
            for c in range(1, 8):
                nc.sync.dma_start(out=xT_s[:, c, :], in_=xTr[c])

            x_s = xp.tile([128, 32, E], f8)
            xr = x8.ap().rearrange("(g r p) e -> g p r e", g=4, p=128)
            for g in range(4):
                nc.sync.dma_start(out=x_s[:, 8 * g : 8 * (g + 1), :], in_=xr[g])

            w1_s = w1p.tile([128, 8, 32, 128], f8)
            w1r = w1.ap().rearrange("(g cp p) (q m) -> g p cp q m", g=4, p=128, m=128)
            for g in range(4):
                nc.sync.dma_start(out=w1_s[:, 2 * g : 2 * (g + 1), :, :], in_=w1r[g])
            wv_s = wvp.tile([128, 8, 8, 128], f16)
            nc.sync.dma_start(
                out=wv_s[:],
                in_=wv.ap().rearrange("(c p) (j m) -> p c j m", p=128, m=128),
            )

            # W2 buffer transfers queue after W1 (FIFO)
            w2_s = w2p.tile([128, 32, 8, 128], f16)
            w2r = w2.ap().rearrange("(g q p) (r m) -> g p q r m", g=8, p=128, m=128)
            for g in range(8):
                nc.sync.dma_start(out=w2_s[:, 4 * g : 4 * (g + 1), :, :], in_=w2r[g])

            # ---- Phases A+A2+B, fused per quarter of N to bound PSUM use.
            # Per quarter Q (slices j=2Q, 2Q+1): accumulate logits over e into
            # attT_q, then per slice: copy to SBUF, transpose per n-tile,
            # exp(+mask bias), and immediately run the y/z accumulation
            # matmuls for those tiles.  Keeps the PE stream dense (HAM warm)
            # and overlaps the exp/copy latency with matmuls.
            psB_cm = tc.tile_pool(name="psB", bufs=1, space="PSUM")
            psB = psB_cm.__enter__()
            psA_cm = tc.tile_pool(name="psA", bufs=3, space="PSUM")
            psA = psA_cm.__enter__()
            psTr_cm = tc.tile_pool(name="psTr", bufs=2, space="PSUM")
            psTr = psTr_cm.__enter__()

            att_n = work.tile([128, 32 * H], f16)
            attm = work.tile([H, N], f16)
            y_ps = psB.tile([H, E], f32, tag="acc")
            z_ps = psB.tile([H, 1], f32, tag="accz")
            attqs = {}

            def emit_A1(j):
                attq = psA.tile([H, 512], f32, tag="attq")
                attqs[j] = attq
                for c in range(8):
                    nc.tensor.matmul(
                        attq[:],
                        watt_s[:, c, :],
                        xT_s[:, c, 512 * j : 512 * (j + 1)],
                        start=(c == 0),
                        stop=(c == 7),
                    )

            # software-pipelined: slice j+1's logit matmuls are emitted before
            # slice j's extraction so the PE overlaps the copy/exp latency
            emit_A1(0)
            emit_A1(1)
            for j in range(8):
                if j + 2 < 8:
                    emit_A1(j + 2)
                sl = slice(512 * j, 512 * (j + 1))
                if j % 2 == 0:
                    nc.vector.tensor_copy(attm[:, sl], attqs[j][:])
                else:
                    nc.scalar.copy(attm[:, sl], attqs[j][:])
                del attqs[j]
                for u in range(4):
                    t = 4 * j + u
                    tr = psTr.tile([128, H], f16, tag="tr")
                    nc.tensor.transpose(
                        tr[:], attm[:, 128 * t : 128 * (t + 1)], identity[:, :]
                    )
                    nc.scalar.activation(
                        att_n[:, H * t : H * (t + 1)],
                        tr[:],
                        Act.Exp,
                        bias=cp_s[:, t : t + 1],
                    )
                for u in range(4):
                    t = 4 * j + u
                    lhs = att_n[:, H * t : H * (t + 1)]
                    nc.tensor.matmul(
                        y_ps[:, 0:512],
                        lhs,
                        x_s[:, t, 0:512],
                        start=(t == 0),
                        stop=(t == 31),
                    )
                    nc.tensor.matmul(
                        y_ps[:, 512:1024],
                        lhs,
                        x_s[:, t, 512:1024],
                        start=(t == 0),
                        stop=(t == 31),
                    )
                    nc.tensor.matmul(
                        z_ps[:],
                        lhs,
                        ones_s[:],
                        start=(t == 0),
                        stop=(t == 31),
                    )
            # normalize: y = y / z
            rz = work.tile([H, 1], f32)
            nc.vector.reciprocal(rz[:], z_ps[:, 0:1])
            y_s = work.tile([H, E], f16)
            nc.vector.tensor_scalar_mul(y_s[:], y_ps[:], rz[:])

            # yT[e, h] chunks (fp16) for phase C
            yT = work.tile([128, 8 * H], f16)
            for j in range(8):
                tr2 = psTr.tile([128, H], f16, tag="tr")
                nc.tensor.transpose(
                    tr2[:], y_s[:, 128 * j : 128 * (j + 1)], identity[:, :]
                )
                nc.vector.tensor_copy(yT[:, H * j : H * (j + 1)], tr2[:])
            psTr_cm.__exit__(None, None, None)
            psA_cm.__exit__(None, None, None)
            psB_cm.__exit__(None, None, None)

            # ---- Phase C: diagonal blocks of sf = y @ Wv, directly transposed.
            # sfT_j[m, i] = sf[2j+i, 128j+m]; s[128j+m] = sfT_j[m, m//64].
            # ---- Phases C+E1 fused: as each s column j lands, immediately
            # run the W1 z-chain contributions for c=j (single-start
            # interleaved accumulation into zT, like the oT chains).
            psC_cm = tc.tile_pool(name="psC", bufs=2, space="PSUM")
            psC = psC_cm.__enter__()
            psM_cm = tc.tile_pool(name="psM", bufs=1, space="PSUM")
            psM = psM_cm.__enter__()
            s_f = work.tile([128, 8], f32)
            s16 = work.tile([128, 8], f16)
            zT_ps = psM.tile([128, 32], f32, tag="z")
            for j in range(8):
                sfT = psC.tile([128, 2], f32, tag="sf")
                for c in range(8):
                    nc.tensor.matmul(
                        sfT[:],
                        wv_s[:, c, j, :],
                        yT[:, 16 * c + 2 * j : 16 * c + 2 * j + 2],
                        start=(c == 0),
                        stop=(c == 7),
                    )
                nc.vector.tensor_copy(s_f[0:64, j : j + 1], sfT[0:64, 0:1])
                nc.vector.tensor_copy(s_f[64:128, j : j + 1], sfT[64:128, 1:2])
                nc.vector.tensor_add(
                    s_f[:, j : j + 1], s_f[:, j : j + 1], cp_s[:, 32 + j : 33 + j]
                )
                nc.vector.tensor_copy(s16[:, j : j + 1], s_f[:, j : j + 1])
                for q in range(32):
                    nc.tensor.matmul(
                        zT_ps[:, q : q + 1],
                        w1_s[:, j, q, :],
                        s16[:, j : j + 1],
                        start=(j == 0 and q == 0),
                        stop=(j == 7 and q == 31),
                        skip_group_check=True,
                    )

            # gelu via sigmoid approximation: z * sigmoid(1.702 z)
            z_s = work.tile([128, 32], f32, tag="zs")
            nc.vector.tensor_add(z_s[:], zT_ps[:], cp_s[:, 40:72])
            sg = work.tile([128, 32], f32, tag="ga")
            nc.scalar.activation(sg[:], z_s[:], Act.Sigmoid, scale=1.702)
            h16 = work.tile([128, 32], f16, tag="h16")
            nc.vector.tensor_mul(h16[:], sg[:], z_s[:])

            # oT chains partial-accumulate per q-group so they pipeline
            # behind the 4 sliced w2 DMAs.  A start flag marks the whole 2KB
            # zero-region pending-zero, so only the very first matmul of the
            # tile may carry it; later first-touches of other columns still
            # overwrite via the lazy pending-zero bytes.
            oT_ps = psM.tile([128, 8], f32, tag="o")
            for g in range(8):
                for r in range(8):
                    for q in range(4 * g, 4 * (g + 1)):
                        nc.tensor.matmul(
                            oT_ps[:, r : r + 1],
                            w2_s[:, q, r, :],
                            h16[:, q : q + 1],
                            start=(g == 0 and r == 0 and q == 0),
                            stop=(g == 7 and r == 7 and q == 31),
                            skip_group_check=True,
                        )

            of = work.tile([128, 8], f32, tag="of")
            nc.vector.tensor_add(of[:], oT_ps[:], cp_s[:, 72:80])
            nc.vector.tensor_add(of[:], of[:], s_f[:])
            nc.sync.dma_start(out=out[:, :], in_=of[:])
            psM_cm.__exit__(None, None, None)
            psC_cm.__exit__(None, None, None)

    return nc


def get_nc():
    if "nc" not in _CACHE:
        nc = _build()
        nc.finalize()
        _CACHE["nc"] = nc
    return _CACHE["nc"]


def build_in_maps(x, mask, W_kv, b_kv, query, W1, b1, W2, b2):
    """Host-side shard prep. Weight-only algebra + layout transforms."""
    import ml_dtypes

    f16 = np.dtype(np.float16)
    f8 = np.dtype(ml_dtypes.float8_e4m3)

    x = np.asarray(x, np.float32)
    mask = np.asarray(mask)
    W_kv = np.asarray(W_kv, np.float32)
    b_kv = np.asarray(b_kv, np.float32)
    query = np.asarray(query, np.float32)
    W1 = np.asarray(W1, np.float32)
    b1 = np.asarray(b1, np.float32)
    W2 = np.asarray(W2, np.float32)
    b2 = np.asarray(b2, np.float32)

    W_k = W_kv[:, :E]
    W_v = W_kv[:, E:]
    # fold the per-head query into the k-projection: [E, H]
    w_att = np.einsum("ehd,hd->eh", W_k.reshape(E, H, D), query).astype(np.float32)

    addmask = np.where(mask[:, :, 0], np.float32(-1e30), np.float32(0.0))  # [B, N]

    watt_c = np.ascontiguousarray(w_att.astype(f16))
    wv_c = np.ascontiguousarray(W_v.astype(f16))
    w1_c = np.ascontiguousarray(W1.astype(f8))
    w2_c = np.ascontiguousarray(W2.astype(f16))

    cpack_base = np.zeros((128, 80), np.float32)
    cpack_base[:, 32:40] = b_kv[E:].reshape(8, 128).T
    cpack_base[:, 40:72] = b1.reshape(32, 128).T
    cpack_base[:, 72:80] = b2.reshape(8, 128).T

    in_maps = []
    for c in range(NCORES):
        cp = cpack_base.copy()
        # maskn[p, t] = addmask[n = 128*t + p]
        cp[:, 0:32] = addmask[c].reshape(32, 128).T
        in_maps.append(
            {
                "xT8": np.ascontiguousarray(x[c].T.astype(f8)),
                "x8": np.ascontiguousarray(x[c].astype(f8)),
                "watt": watt_c,
                "cpack": cp,
                "wv": wv_c,
                "w1": w1_c,
                "w2": w2_c,
            }
        )
    return in_maps


def kernel(**inputs):
    from concourse.bass_utils import run_bass_kernel_spmd

    in_maps = build_in_maps(**inputs)
    nc = get_nc()
    res = run_bass_kernel_spmd(nc, in_maps, list(range(NCORES)), trace=False)
    # out is [128, 8] with out_row[128*j + p] = out[p, j]
    return np.stack(
        [np.asarray(res.results[c]["out"]).T.reshape(-1) for c in range(NCORES)]
    ).astype(np.float32)


# revision 30
# speedup vs baseline: 1.0680x; 1.0680x over previous
"""Trainium2 Bass kernel for nn_AttentionToVec (B=8, N=4096, E=1024, H=16, D=64).

Strategy: pure data-parallel over batch (1 batch element per NeuronCore), NO
collectives.  Each core computes its own row's full MLP with the complete
W1/W2 (profiling showed the AllGather/ReduceScatter + cc-barrier of the
tensor-parallel MLP cost ~100us, far more than the extra weight traffic).

DMA discipline: all loads go through the single HWDGE sync queue, which
drains FIFO in issue order.  Every stream tensor is fully resident in SBUF
(no pool-buffer gating), so the issue order IS the arrival order:
  xT (fp8, 4.2MB) -> x (fp8, 4.2MB) -> Wv (fp16, 2.1MB) -> W1 (fp8, 4.2MB)
  -> W2 (fp16, 8.4MB, sliced so the W2 matmuls pipeline behind arrival).
The W2 buffer reuses the xT pool's SBUF (xT pool closes after phase A).

Dtypes (validated vs reference on host, rel-err ~1.25e-2 vs 2e-2 budget):
  xT/x fp8e4m3 (mixed with fp16 operands), W1 fp8e4m3 (mixed with fp16
  moving s), everything else fp16; all matmul accumulation fp32 in PSUM.

Algebra (host does weight-only folding):
  - att logits = x @ w_att,  w_att[e,h] = sum_d W_k[e, h*D+d] * query[h,d]
    (the k-projection bias cancels inside softmax over n).
  - y[h,:] = sum_n exp_att[n,h] * x[n,:]  (deferred 1/Z normalization)
  - sampled[e] = (y[h(e),:] @ W_v[:, e]) + b_v[e],  h(e)=e//D.  Phase C
    computes ONLY the needed diagonal blocks, directly transposed:
    sfT_j[m, i] = sf[2j+i, 128j+m] so s[128j+m] = sfT_j[m, m//64].
  - MLP per-core on its own row, hidden laid out as zT[p, q] = z[128q+p]
    so gelu runs across all 128 partitions.
"""

import numpy as np

B = 8
N = 4096
E = 1024
H = 16
D = 64
HID = 4096
NCORES = 8

_CACHE = {}


def _build():
    import concourse.bacc as bacc
    import concourse.mybir as mybir
    from concourse import tile
    from concourse.masks import make_identity

    f32 = mybir.dt.float32
    f16 = mybir.dt.float16
    f8 = mybir.dt.float8e4
    Act = mybir.ActivationFunctionType
    Alu = mybir.AluOpType

    nc = bacc.Bacc(None, target_bir_lowering=False, debug=True, num_devices=NCORES)

    xT8 = nc.dram_tensor("xT8", [E, N], f8, kind="ExternalInput")
    x8 = nc.dram_tensor("x8", [N, E], f8, kind="ExternalInput")
    watt = nc.dram_tensor("watt", [E, H], f16, kind="ExternalInput")
    # packed [128, 80] f32: cols 0:32 maskn, 32:40 bvT, 40:72 b1T, 72:80 b2T
    cpack = nc.dram_tensor("cpack", [128, 80], f32, kind="ExternalInput")
    wv = nc.dram_tensor("wv", [E, E], f16, kind="ExternalInput")
    w1 = nc.dram_tensor("w1", [E, HID], f8, kind="ExternalInput")
    w2 = nc.dram_tensor("w2", [HID, E], f16, kind="ExternalInput")
    # outT[p, j] = out_row[128*j + p]; host reassembles
    out = nc.dram_tensor("out", [128, 8], f32, kind="ExternalOutput")

    with tile.TileContext(nc) as tc:
        with (
            tc.tile_pool(name="consts", bufs=1) as consts,
            tc.tile_pool(name="xp", bufs=1) as xp,
            tc.tile_pool(name="xtp", bufs=1) as xtp,
            tc.tile_pool(name="wvp", bufs=1) as wvp,
            tc.tile_pool(name="w2p", bufs=1) as w2p,
            tc.tile_pool(name="w1p", bufs=1) as w1p,
            tc.tile_pool(name="work", bufs=1) as work,
        ):
            identity = consts.tile([H, H], f16)
            make_identity(nc, identity[:])
            ones_s = consts.tile([128, 1], f16)
            nc.vector.memset(ones_s[:], 1.0)

            # ---- all DMA triggers in FIFO priority order ----
            watt_s = consts.tile([128, 8, H], f16)
            nc.sync.dma_start(
                out=watt_s[:], in_=watt.ap().rearrange("(c p) h -> p c h", p=128)
            )
            cp_s = consts.tile([128, 80], f32)
            nc.sync.dma_start(out=cp_s[:], in_=cpack[:, :])

            xT_s = xtp.tile([128, 8, N], f8)
            xTr = xT8.ap().rearrange("(c p) n -> c p n", p=128)
            nc.sync.dma_start(out=xT_s[:, 0, 0:2048], in_=xTr[0][:, 0:2048])
            nc.sync.dma_start(out=xT_s[:, 0, 2048:4096], in_=xTr[0][:, 2048:4096])
            for c in range(1, 8):
                nc.sync.dma_start(out=xT_s[:, c, :], in_=xTr[c])

            x_s = xp.tile([128, 32, E], f8)
            xr = x8.ap().rearrange("(g r p) e -> g p r e", g=4, p=128)
            for g in range(4):
                nc.sync.dma_start(out=x_s[:, 8 * g : 8 * (g + 1), :], in_=xr[g])

            w1_s = w1p.tile([128, 8, 32, 128], f8)
            w1r = w1.ap().rearrange("(g cp p) (q m) -> g p cp q m", g=4, p=128, m=128)
            for g in range(4):
                nc.sync.dma_start(out=w1_s[:, 2 * g : 2 * (g + 1), :, :], in_=w1r[g])
            wv_s = wvp.tile([128, 8, 8, 128], f16)
            nc.sync.dma_start(
                out=wv_s[:],
                in_=wv.ap().rearrange("(c p) (j m) -> p c j m", p=128, m=128),
            )

            # W2 buffer transfers queue after W1 (FIFO)
            w2_s = w2p.tile([128, 32, 8, 128], f16)
            w2r = w2.ap().rearrange("(g q p) (r m) -> g p q r m", g=8, p=128, m=128)
            for g in range(8):
                nc.sync.dma_start(out=w2_s[:, 4 * g : 4 * (g + 1), :, :], in_=w2r[g])

            # ---- Phases A+A2+B, fused per quarter of N to bound PSUM use.
            # Per quarter Q (slices j=2Q, 2Q+1): accumulate logits over e into
            # attT_q, then per slice: copy to SBUF, transpose per n-tile,
            # exp(+mask bias), and immediately run the y/z accumulation
            # matmuls for those tiles.  Keeps the PE stream dense (HAM warm)
            # and overlaps the exp/copy latency with matmuls.
            psB_cm = tc.tile_pool(name="psB", bufs=1, space="PSUM")
            psB = psB_cm.__enter__()
            psA_cm = tc.tile_pool(name="psA", bufs=2, space="PSUM")
            psA = psA_cm.__enter__()
            psTr_cm = tc.tile_pool(name="psTr", bufs=3, space="PSUM")
            psTr = psTr_cm.__enter__()

            att_n = work.tile([128, 32 * H], f16)
            attm = work.tile([H, N], f16)
            y_ps = psB.tile([H, E], f32, tag="acc")
            z_ps = psB.tile([H, 1], f32, tag="accz")
            attqs = {}

            def emit_A1(j):
                attq = psA.tile([H, 512], f32, tag="attq")
                attqs[j] = attq
                for c in range(8):
                    nc.tensor.matmul(
                        attq[:],
                        watt_s[:, c, :],
                        xT_s[:, c, 512 * j : 512 * (j + 1)],
                        start=(c == 0),
                        stop=(c == 7),
                    )

            # software-pipelined: slice j+1's logit matmuls are emitted before
            # slice j's extraction so the PE overlaps the copy/exp latency
            emit_A1(0)
            for j in range(8):
                if j + 1 < 8:
                    emit_A1(j + 1)
                sl = slice(512 * j, 512 * (j + 1))
                if j % 2 == 0:
                    nc.vector.tensor_copy(attm[:, sl], attqs[j][:])
                else:
                    nc.scalar.copy(attm[:, sl], attqs[j][:])
                del attqs[j]
                for u in range(4):
                    t = 4 * j + u
                    tr = psTr.tile([128, H], f16, tag="tr")
                    nc.tensor.transpose(
                        tr[:], attm[:, 128 * t : 128 * (t + 1)], identity[:, :]
                    )
                    nc.scalar.activation(
                        att_n[:, H * t : H * (t + 1)],
                        tr[:],
                        Act.Exp,
                        bias=cp_s[:, t : t + 1],
                    )
                for u in range(4):
                    t = 4 * j + u
                    lhs = att_n[:, H * t : H * (t + 1)]
                    nc.tensor.matmul(
                        y_ps[:, 0:512],
                        lhs,
                        x_s[:, t, 0:512],
                        start=(t == 0),
                        stop=(t == 31),
                    )
                    nc.tensor.matmul(
                        y_ps[:, 512:1024],
                        lhs,
                        x_s[:, t, 512:1024],
                        start=(t == 0),
                        stop=(t == 31),
                    )
                    nc.tensor.matmul(
                        z_ps[:],
                        lhs,
                        ones_s[:],
                        start=(t == 0),
                        stop=(t == 31),
                    )
            # normalize: y = y / z
            rz = work.tile([H, 1], f32)
            nc.vector.reciprocal(rz[:], z_ps[:, 0:1])
            y_s = work.tile([H, E], f16)
            nc.vector.tensor_scalar_mul(y_s[:], y_ps[:], rz[:])

            # yT[e, h] chunks (fp16) for phase C
            yT = work.tile([128, 8 * H], f16)
            for j in range(8):
                tr2 = psTr.tile([128, H], f16, tag="tr")
                nc.tensor.transpose(
                    tr2[:], y_s[:, 128 * j : 128 * (j + 1)], identity[:, :]
                )
                nc.vector.tensor_copy(yT[:, H * j : H * (j + 1)], tr2[:])
            psTr_cm.__exit__(None, None, None)
            psA_cm.__exit__(None, None, None)
            psB_cm.__exit__(None, None, None)

            # ---- Phase C: diagonal blocks of sf = y @ Wv, directly transposed.
            # sfT_j[m, i] = sf[2j+i, 128j+m]; s[128j+m] = sfT_j[m, m//64].
            # ---- Phases C+E1 fused: as each s column j lands, immediately
            # run the W1 z-chain contributions for c=j (single-start
            # interleaved accumulation into zT, like the oT chains).
            psC_cm = tc.tile_pool(name="psC", bufs=2, space="PSUM")
            psC = psC_cm.__enter__()
            psM_cm = tc.tile_pool(name="psM", bufs=1, space="PSUM")
            psM = psM_cm.__enter__()
            s_f = work.tile([128, 8], f32)
            s16 = work.tile([128, 8], f16)
            zT_ps = psM.tile([128, 32], f32, tag="z")
            for j in range(8):
                sfT = psC.tile([128, 2], f32, tag="sf")
                for c in range(8):
                    nc.tensor.matmul(
                        sfT[:],
                        wv_s[:, c, j, :],
                        yT[:, 16 * c + 2 * j : 16 * c + 2 * j + 2],
                        start=(c == 0),
                        stop=(c == 7),
                    )
                nc.vector.tensor_copy(s_f[0:64, j : j + 1], sfT[0:64, 0:1])
                nc.vector.tensor_copy(s_f[64:128, j : j + 1], sfT[64:128, 1:2])
                nc.vector.tensor_add(
                    s_f[:, j : j + 1], s_f[:, j : j + 1], cp_s[:, 32 + j : 33 + j]
                )
                nc.vector.tensor_copy(s16[:, j : j + 1], s_f[:, j : j + 1])
                for q in range(32):
                    nc.tensor.matmul(
                        zT_ps[:, q : q + 1],
                        w1_s[:, j, q, :],
                        s16[:, j : j + 1],
                        start=(j == 0 and q == 0),
                        stop=(j == 7 and q == 31),
                        skip_group_check=True,
                    )

            # gelu via sigmoid approximation: z * sigmoid(1.702 z)
            z_s = work.tile([128, 32], f32, tag="zs")
            nc.vector.tensor_add(z_s[:], zT_ps[:], cp_s[:, 40:72])
            sg = work.tile([128, 32], f32, tag="ga")
            nc.scalar.activation(sg[:], z_s[:], Act.Sigmoid, scale=1.702)
            h16 = work.tile([128, 32], f16, tag="h16")
            nc.vector.tensor_mul(h16[:], sg[:], z_s[:])

            # oT chains partial-accumulate per q-group so they pipeline
            # behind the 4 sliced w2 DMAs.  A start flag marks the whole 2KB
            # zero-region pending-zero, so only the very first matmul of the
            # tile may carry it; later first-touches of other columns still
            # overwrite via the lazy pending-zero bytes.
            oT_ps = psM.tile([128, 8], f32, tag="o")
            for g in range(8):
                for r in range(8):
                    for q in range(4 * g, 4 * (g + 1)):
                        nc.tensor.matmul(
                            oT_ps[:, r : r + 1],
                            w2_s[:, q, r, :],
                            h16[:, q : q + 1],
                            start=(g == 0 and r == 0 and q == 0),
                            stop=(g == 7 and r == 7 and q == 31),
                            skip_group_check=True,
                        )

            of = work.tile([128, 8], f32, tag="of")
            nc.vector.tensor_add(of[:], oT_ps[:], cp_s[:, 72:80])
            nc.vector.tensor_add(of[:], of[:], s_f[:])
            nc.sync.dma_start(out=out[:, :], in_=of[:])
            psM_cm.__exit__(None, None, None)
            psC_cm.__exit__(None, None, None)

    return nc


def get_nc():
    if "nc" not in _CACHE:
        nc = _build()
        nc.finalize()
        _CACHE["nc"] = nc
    return _CACHE["nc"]


def build_in_maps(x, mask, W_kv, b_kv, query, W1, b1, W2, b2):
    """Host-side shard prep. Weight-only algebra + layout transforms."""
    import ml_dtypes

    f16 = np.dtype(np.float16)
    f8 = np.dtype(ml_dtypes.float8_e4m3)

    x = np.asarray(x, np.float32)
    mask = np.asarray(mask)
    W_kv = np.asarray(W_kv, np.float32)
    b_kv = np.asarray(b_kv, np.float32)
    query = np.asarray(query, np.float32)
    W1 = np.asarray(W1, np.float32)
    b1 = np.asarray(b1, np.float32)
    W2 = np.asarray(W2, np.float32)
    b2 = np.asarray(b2, np.float32)

    W_k = W_kv[:, :E]
    W_v = W_kv[:, E:]
    # fold the per-head query into the k-projection: [E, H]
    w_att = np.einsum("ehd,hd->eh", W_k.reshape(E, H, D), query).astype(np.float32)

    addmask = np.where(mask[:, :, 0], np.float32(-1e30), np.float32(0.0))  # [B, N]

    watt_c = np.ascontiguousarray(w_att.astype(f16))
    wv_c = np.ascontiguousarray(W_v.astype(f16))
    w1_c = np.ascontiguousarray(W1.astype(f8))
    w2_c = np.ascontiguousarray(W2.astype(f16))

    cpack_base = np.zeros((128, 80), np.float32)
    cpack_base[:, 32:40] = b_kv[E:].reshape(8, 128).T
    cpack_base[:, 40:72] = b1.reshape(32, 128).T
    cpack_base[:, 72:80] = b2.reshape(8, 128).T

    in_maps = []
    for c in range(NCORES):
        cp = cpack_base.copy()
        # maskn[p, t] = addmask[n = 128*t + p]
        cp[:, 0:32] = addmask[c].reshape(32, 128).T
        in_maps.append(
            {
                "xT8": np.ascontiguousarray(x[c].T.astype(f8)),
                "x8": np.ascontiguousarray(x[c].astype(f8)),
                "watt": watt_c,
                "cpack": cp,
                "wv": wv_c,
                "w1": w1_c,
                "w2": w2_c,
            }
        )
    return in_maps


def kernel(**inputs):
    from concourse.bass_utils import run_bass_kernel_spmd

    in_maps = build_in_maps(**inputs)
    nc = get_nc()
    res = run_bass_kernel_spmd(nc, in_maps, list(range(NCORES)), trace=False)
    # out is [128, 8] with out_row[128*j + p] = out[p, j]
    return np.stack(
        [np.asarray(res.results[c]["out"]).T.reshape(-1) for c in range(NCORES)]
    ).astype(np.float32)
